# revision 20
# baseline (speedup 1.0000x reference)
"""MoE router (group-limited greedy top-k) Trainium2 Bass kernel.

Problem: nn_MoERouter_24283745091734
  x [16384, 2048] f32, W [256, 2048] f32, e_bias [256] f32 (zeros)
  logits = x @ W.T ; scores = sigmoid(logits)
  group-limited routing: G=8 groups of 32 experts, group score = sum of
  top-2 biased scores in group, keep top-4 groups, then top-8 experts
  overall; weights = unbiased scores at top-8, normalized.

Sharding: tokens (S) split across 8 cores, 2048 tokens each. W/bias
replicated. Each core:
  - x slice host-transposed to xT [2048(D), 2048(S)] so the tensor engine
    can contract over D on partitions with tokens on PSUM partitions.
  - 16 token tiles of 128; per tile 16x3 accumulating float32r matmuls
    into PSUM [128, 256] (hi/lo 12-bit split of x and W: xh@Wh + xh@Wl +
    xl@Wh gives fp32-quality logits at ~1 PE cycle/row instead of native
    fp32's 4), sigmoid on ScalarE, then the routing on the vector engine
    using the top-8 custom ops (max / max_index / match_replace).
"""

import os

import numpy as np

import concourse.bass as bass
import concourse.mybir as mybir
import concourse.tile as tile
from concourse import bacc
from concourse.bass_utils import run_bass_kernel_spmd

S, D, E, G, K, TKG = 16384, 2048, 256, 8, 8, 4
EPG = E // G  # 32
N_CORES = 8
S_PER = S // N_CORES  # 2048 tokens per core
P = 128
NT = S_PER // P  # 16 token tiles per core
KT = D // P  # 16 contraction tiles

_PROG_CACHE: dict = {}
LAST_RESULTS = None


def _build_program(
    mm_dtype_name: str,
    *,
    do_matmul: bool = True,
    do_routing: bool = True,
    x_bufs: int = 3,
    ps_bufs: int = 4,
    sc_bufs: int = 4,
):
    f32 = mybir.dt.float32
    f32r = mybir.dt.float32r
    f32r3 = mm_dtype_name == "f32r3"
    mm_dt = f32 if f32r3 else getattr(mybir.dt, mm_dtype_name)

    nc = bacc.Bacc("TRN2", target_bir_lowering=False, debug=False)

    xT = nc.dram_tensor("xT", [D, S_PER], f32, kind="ExternalInput").ap()
    WT = nc.dram_tensor("WT", [D, E], f32, kind="ExternalInput").ap()
    scores_o = nc.dram_tensor("scores", [S_PER, E], f32, kind="ExternalOutput").ap()
    idx_o = nc.dram_tensor(
        "topk_idx", [S_PER, K], mybir.dt.uint32, kind="ExternalOutput"
    ).ap()
    wts_o = nc.dram_tensor("weights", [S_PER, K], f32, kind="ExternalOutput").ap()

    xT_view = xT.rearrange("(k p) s -> p k s", p=P)  # [128, 16, 2048]
    WT_view = WT.rearrange("(k p) e -> p k e", p=P)  # [128, 16, 256]

    with tile.TileContext(nc) as tc:
        with (
            tc.tile_pool(name="const", bufs=1) as const_pool,
            tc.tile_pool(name="xin", bufs=x_bufs) as xpool,
            tc.tile_pool(name="psum", bufs=ps_bufs, space="PSUM") as pspool,
            tc.tile_pool(name="scores", bufs=sc_bufs) as scpool,
            tc.tile_pool(name="wide", bufs=4) as wide_pool,
            tc.tile_pool(name="small", bufs=4) as small_pool,
            tc.tile_pool(name="acc", bufs=1) as acc_pool,
        ):
            vals_all = acc_pool.tile([P, NT, K], f32)
            idx_all = acc_pool.tile([P, NT, K], mybir.dt.uint32)

            wt_sb = const_pool.tile([P, KT, E], f32)
            if f32r3:
                wh_sb = const_pool.tile([P, KT, E], f32r)
                wl_sb = const_pool.tile([P, KT, E], f32r)

            pending = None
            for t in range(NT):
                ts = slice(t * P, (t + 1) * P)

                xt = xpool.tile([P, KT, P], f32, tag="xt")
                if t == 0:
                    # interleave the W-tile loads with tile-0 x chunks so the
                    # first matmuls are not queued behind the whole W load
                    for lo, hi in ((0, 4), (4, 8), (8, 12), (12, 16)):
                        nc.sync.dma_start(
                            xt[:, lo:hi, :], xT_view[:, lo:hi, ts]
                        )
                        for k in range(lo, hi):
                            nc.sync.dma_start(wt_sb[:, k, :], WT_view[:, k, :])
                else:
                    for kc in range(0, KT, 4):
                        nc.sync.dma_start(
                            xt[:, kc : kc + 4, :], xT_view[:, kc : kc + 4, ts]
                        )

                chunks = [(0, 4), (4, 8), (8, 12), (12, 16)]
                if f32r3 and t == 0:
                    # one-time W hi/lo split (f32r keeps ~12 mantissa bits),
                    # chunked so the first matmuls are not gated on all of it
                    for lo, hi in chunks:
                        kcs = slice(lo, hi)
                        nc.scalar.activation(
                            wh_sb[:, kcs, :], wt_sb[:, kcs, :],
                            mybir.ActivationFunctionType.Copy,
                        )
                        nc.vector.tensor_sub(
                            wl_sb[:, kcs, :], wt_sb[:, kcs, :],
                            wh_sb[:, kcs, :].bitcast(f32),
                        )

                if f32r3:
                    xh = xpool.tile([P, KT, P], f32r, tag="xh")
                    xl = xpool.tile([P, KT, P], f32r, tag="xl")
                    for lo, hi in chunks:
                        kcs = slice(lo, hi)
                        nc.scalar.activation(
                            xh[:, kcs, :], xt[:, kcs, :],
                            mybir.ActivationFunctionType.Copy,
                        )
                        nc.vector.tensor_sub(
                            xl[:, kcs, :], xt[:, kcs, :],
                            xh[:, kcs, :].bitcast(f32),
                        )

                ps = pspool.tile([P, E], f32, tag="ps")
                if do_matmul and f32r3:
                    n = 0
                    for k in range(KT):
                        for lhs, rhs in (
                            (xh[:, k, :], wh_sb[:, k, :]),
                            (xh[:, k, :], wl_sb[:, k, :]),
                            (xl[:, k, :], wh_sb[:, k, :]),
                        ):
                            nc.tensor.matmul(
                                ps[:], lhsT=lhs, rhs=rhs,
                                start=(n == 0), stop=(n == 3 * KT - 1),
                            )
                            n += 1
                elif do_matmul:
                    for k in range(KT):
                        nc.tensor.matmul(
                            ps[:],
                            lhsT=xt[:, k, :].bitcast(mm_dt),
                            rhs=wt_sb[:, k, :].bitcast(mm_dt),
                            start=(k == 0),
                            stop=(k == KT - 1),
                        )
                else:
                    nc.vector.memset(ps[:, :1], 0.0)

                sc = scpool.tile([P, E], f32, tag="sc")
                nc.scalar.activation(
                    sc[:], ps[:], mybir.ActivationFunctionType.Sigmoid
                )
                nc.scalar.dma_start(scores_o[ts, :], sc[:])

                # ---- routing (emitted one tile late so the DVE prioritizes
                # the hi/lo split feeding the tensor engine) ----
                def do_route(t, sc):
                    sc_g = sc[:].rearrange("p (g e) -> p g e", g=G)
                    g1 = small_pool.tile([P, G], f32, tag="g1")
                    nc.vector.tensor_reduce(
                        g1[:], sc_g, axis=mybir.AxisListType.X,
                        op=mybir.AluOpType.max,
                    )
                    repl = wide_pool.tile([P, E], f32, tag="repl")
                    nc.vector.match_replace(
                        out=repl[:], in_to_replace=g1[:], in_values=sc[:],
                        imm_value=-1e30,
                    )
                    g2 = small_pool.tile([P, G], f32, tag="g2")
                    nc.vector.tensor_reduce(
                        g2[:],
                        repl[:].rearrange("p (g e) -> p g e", g=G),
                        axis=mybir.AxisListType.X,
                        op=mybir.AluOpType.max,
                    )
                    gsum = small_pool.tile([P, G], f32, tag="gsum")
                    nc.vector.tensor_add(gsum[:], g1[:], g2[:])
                    gs8 = small_pool.tile([P, 8], f32, tag="gs8")
                    nc.vector.max(out=gs8[:], in_=gsum[:])
                    masked = wide_pool.tile([P, E], f32, tag="masked")
                    nc.vector.scalar_tensor_tensor(
                        out=masked[:].rearrange("p (g e) -> p g e", g=G),
                        in0=gsum[:].unsqueeze(2).to_broadcast([P, G, EPG]),
                        scalar=gs8[:, 3:4],
                        in1=sc_g,
                        op0=mybir.AluOpType.is_ge,
                        op1=mybir.AluOpType.mult,
                    )
                    vals = vals_all[:, t, :]
                    nc.vector.max(out=vals, in_=masked[:])
                    idx = idx_all[:, t, :]
                    nc.vector.max_index(out=idx, in_max=vals, in_values=masked[:])

                if do_routing:
                    if pending is not None:
                        do_route(*pending)
                    pending = (t, sc)

            # ---- batched weight normalization + index/weight stores ----
            if do_routing:
                if pending is not None:
                    do_route(*pending)
                wsum = acc_pool.tile([P, NT], f32)
                nc.vector.tensor_reduce(
                    wsum[:], vals_all[:], axis=mybir.AxisListType.X,
                    op=mybir.AluOpType.add,
                )
                rec = acc_pool.tile([P, NT], f32)
                nc.vector.reciprocal(rec[:], wsum[:])
                wts = acc_pool.tile([P, NT, K], f32)
                nc.vector.scalar_tensor_tensor(
                    out=wts[:],
                    in0=rec[:].unsqueeze(2).to_broadcast([P, NT, K]),
                    scalar=1.0,
                    in1=vals_all[:],
                    op0=mybir.AluOpType.mult,
                    op1=mybir.AluOpType.mult,
                )
                # DRAM [S_PER, K] viewed [128p, NT, K]: token = t*128 + p
                idx_o_v = idx_o.rearrange("(t p) k -> p t k", p=P)
                wts_o_v = wts_o.rearrange("(t p) k -> p t k", p=P)
                nc.sync.dma_start(idx_o_v, idx_all[:])
                nc.sync.dma_start(wts_o_v, wts[:])

    nc.compile()
    return nc


def _routing_numpy(scores: np.ndarray, e_bias: np.ndarray):
    """Host fallback for the (never exercised here) nonzero-bias case."""
    s = scores + e_bias
    grp = s.reshape(-1, G, EPG)
    top2 = np.sort(grp, axis=-1)[:, :, -2:].sum(-1)
    gidx = np.argsort(-top2, kind="stable", axis=-1)[:, :TKG]
    mask = np.zeros((s.shape[0], G), dtype=bool)
    np.put_along_axis(mask, gidx, True, axis=1)
    smask = np.repeat(mask, EPG, axis=1)
    sm = np.where(smask, s, -np.inf)
    topk = np.argsort(-sm, kind="stable", axis=-1)[:, :K].astype(np.int32)
    w = np.take_along_axis(scores, topk, axis=1)
    w = w / (w.sum(-1, keepdims=True) + 1e-20)
    return topk, w.astype(np.float32)


def kernel(x: np.ndarray, W: np.ndarray, e_bias: np.ndarray):
    mm_dtype_name = os.environ.get("MOE_MM_DTYPE", "f32r3")

    x = np.ascontiguousarray(x, dtype=np.float32)
    W = np.ascontiguousarray(W, dtype=np.float32)
    e_bias = np.asarray(e_bias, dtype=np.float32)
    assert x.shape == (S, D) and W.shape == (E, D), (x.shape, W.shape)

    key = mm_dtype_name
    if key not in _PROG_CACHE:
        _PROG_CACHE[key] = _build_program(mm_dtype_name)
    nc = _PROG_CACHE[key]

    WT = np.ascontiguousarray(W.T)  # [D, E]
    in_maps = []
    for c in range(N_CORES):
        xc = x[c * S_PER : (c + 1) * S_PER]  # [S_PER, D]
        in_maps.append({"xT": np.ascontiguousarray(xc.T), "WT": WT})

    trace = bool(int(os.environ.get("MOE_TRACE", "0")))
    res = run_bass_kernel_spmd(
        nc, in_maps, core_ids=list(range(N_CORES)), trace=trace
    )
    global LAST_RESULTS
    LAST_RESULTS = res

    scores = np.concatenate([r["scores"] for r in res.results], axis=0)
    if np.any(e_bias != 0.0):
        topk_idx, weights = _routing_numpy(scores, e_bias)
    else:
        topk_idx = np.concatenate(
            [r["topk_idx"].astype(np.int32) for r in res.results], axis=0
        )
        weights = np.concatenate([r["weights"] for r in res.results], axis=0)
    return topk_idx, weights, scores
